# revision 1
# baseline (speedup 1.0000x reference)
"""DeepseekV3 decoder layer on 8 TRN2 NeuronCores.

Sharding: pure data parallel over tokens, zero collectives. B=2, S=1024 ->
2048 tokens; core = (batch b, quarter c) owns 256 query tokens. Each core
recomputes the full-batch KV path (~+10% FLOPs) so attention needs no
cross-core traffic; host assembles the 8 (2048, 256) output slices.

Device kernel: feature-major activations (feat on partitions, tokens on the
free dim) for every matmul; bf16 weights/operands with f32 PSUM accumulation;
RMS-norm partition reductions via Square + ones-matmul (float32r); rope via
host-side column permutation (deinterleave) + duplicated swapped columns so
rotate_half needs no cross-partition moves; softmax without max subtraction
(scores are O(1) by construction); scores computed transposed (tk, tq) so
attn@v contracts tk on partitions for both operands.
"""
import numpy as np
import ml_dtypes

import concourse.bass as bass
import concourse.mybir as mybir
import concourse.tile as tile
from concourse import bacc
from concourse import bass_utils

F32 = mybir.dt.float32
BF16 = mybir.dt.bfloat16
F32R = mybir.dt.float32r
AF = mybir.ActivationFunctionType

H, NH, QLR, KVLR = 2048, 16, 1536, 512
NOPE, ROPE, VD = 128, 64, 128
QHD = NOPE + ROPE
I, B, S = 8192, 2, 1024
EPS = 1e-6
SCALE = QHD ** -0.5
N_CORES = 8
TQ = 256   # query tokens per core
TK = 1024  # key tokens (full batch) per core

bf16 = ml_dtypes.bfloat16


# ---------------------------------------------------------------- device ---

def build_nc():
    from contextlib import ExitStack

    nc = bacc.Bacc("TRN2", target_bir_lowering=False, debug=False)

    d = {}

    def din(name, shape, dt=F32):
        d[name] = nc.dram_tensor(name, shape, dt, kind="ExternalInput").ap()

    din("xkB", (H, TK), BF16)           # raw hidden^T (full batch), bf16
    din("xqB", (H, TQ), BF16)           # raw hidden^T (query slice), bf16
    din("xqT", (H, TQ))                 # residual, f32
    din("cos_kT", (ROPE, TK))
    din("sin_kT", (ROPE, TK))
    din("cos_qT", (ROPE, TQ))
    din("sin_qT", (ROPE, TQ))
    din("maskT", (TK, TQ))
    din("w_qa", (H, QLR), BF16)
    din("w_qb", (QLR, 4096), BF16)      # [nope 16x128 | rope 16x64 | rope_swap 16x64]
    din("w_kva", (H, 640), BF16)        # [lat 512 | pe 64 | pe_swap 64]
    din("w_kvb", (KVLR, 4096), BF16)    # [k_nope 16x128 | v 16x128]
    din("w_o", (H, H), BF16)
    din("w_gate", (H, I), BF16)
    din("w_up", (H, I), BF16)
    din("w_down", (I, H), BF16)
    out_d = nc.dram_tensor("out", (H, TQ), F32, kind="ExternalOutput").ap()

    with tile.TileContext(nc) as tc, ExitStack() as ctx:
        pl0 = ctx.enter_context(tc.tile_pool(name="pl0", bufs=1))
        pw = ctx.enter_context(tc.tile_pool(name="wslab", bufs=3))
        ph1 = ctx.enter_context(tc.tile_pool(name="ph1", bufs=1))      # h1/h1n (E..F)
        pxqf = ctx.enter_context(tc.tile_pool(name="pxqf", bufs=1))    # xqf f32 (0..E)
        pattn = ctx.enter_context(tc.tile_pool(name="pattn", bufs=1))  # attn_out (D..E)
        pkv = ctx.enter_context(tc.tile_pool(name="pkv", bufs=1))      # kv products (B..D)
        pq = ctx.enter_context(tc.tile_pool(name="pq", bufs=1))        # q products (C..D)
        pkv_r = pkv  # r-vectors resident alongside kv products
        pxb = ctx.enter_context(tc.tile_pool(name="pxb", bufs=1))      # xkB/xqB resident
        pmm = ctx.enter_context(tc.tile_pool(name="pmm", bufs=6, space="PSUM"))
        pst = ctx.enter_context(tc.tile_pool(name="pst", bufs=2, space="PSUM"))

        def mktile(pool, shape, dtype, tag):
            return pool.tile(shape, dtype, tag=tag, name=tag)

        ones_b = mktile(pl0, [128, 1], BF16, "ones_b")
        nc.vector.memset(ones_b, 1.0)
        ones_f = mktile(pl0, [128, 1], F32, "ones_f")
        nc.vector.memset(ones_f, 1.0)
        eps_t = mktile(pl0, [1, 1], F32, "eps_t")
        nc.vector.memset(eps_t, EPS)

        # raw activations, bf16, feature-major (resident). Bulk input loads
        # ride the Activation engine's HWDGE queue so weight-slab DMAs (SP
        # queue) are not stuck behind them.
        xkb = [mktile(pxb, [128, TK], BF16, f"xkb{k}") for k in range(16)]
        xqf = [mktile(pxqf, [128, TQ], F32, f"xqf{k}") for k in range(16)]

        # ---------------- generic streamed projection ----------------
        def proj(w_ap, Kt, Mt, rhs_tiles, T, consume, bm=4, kg=4,
                 first_small=False):
            """psum[m, c] = sum_k W[k,m-slice].T @ rhs[k][:, c-slice].

            One psum bank per (m, c) unit (a 2KB psum zero-region admits only
            one pending accumulation group). Weight DMAs fetch kg k-tiles per
            transfer via a 3D access pattern to amortize the ~625ns HWDGE
            fixed cost per dma_start.
            """
            nchunk = max(1, T // 512)
            N = T // nchunk
            for m0 in range(0, Mt, bm):
                ms = list(range(m0, min(m0 + bm, Mt)))
                bw = len(ms) * 128
                units = [(m, c) for m in ms for c in range(nchunk)]
                psap = {}
                for (m, c) in units:
                    psap[(m, c)] = mktile(pmm, [128, N], F32, "mm")
                if first_small and m0 == 0:
                    # smaller leading k-groups so the first slab lands fast
                    groups = [(0, 1), (1, 1)]
                    k0_ = 2
                    while k0_ < Kt:
                        nk_ = min(kg, Kt - k0_)
                        groups.append((k0_, nk_))
                        k0_ += nk_
                else:
                    groups = [(k0_, min(kg, Kt - k0_))
                              for k0_ in range(0, Kt, kg)]
                for k0, nk in groups:
                    wsl = pw.tile([128, nk * bw], BF16, tag="wsl", name="wsl")
                    src = w_ap[k0 * 128:(k0 + nk) * 128,
                               m0 * 128:m0 * 128 + bw]
                    nc.sync.dma_start(
                        out=wsl.rearrange("p (t m) -> p t m", t=nk),
                        in_=src.rearrange("(t p) m -> p t m", p=128))
                    for dk in range(nk):
                        k = k0 + dk
                        st = (k == 0)
                        sp = (k == Kt - 1)
                        for mi, m in enumerate(ms):
                            for c in range(nchunk):
                                nc.tensor.matmul(
                                    psap[(m, c)],
                                    wsl[:, (dk * len(ms) + mi) * 128:
                                        (dk * len(ms) + mi + 1) * 128],
                                    rhs_tiles[k][:, c * N:(c + 1) * N],
                                    start=st, stop=sp)
                for (m, c) in units:
                    consume(m, c, psap[(m, c)])

        def rms_finish(pool, st_tiles, T, nfeat, tag):
            """r = 1/sqrt(sumsq/nfeat + eps): returns ([1,T] row, [128,T]
            partition-replicated)."""
            r = mktile(pool, [1, T], F32, f"r_{tag}")
            nch = len(st_tiles)
            n = T // nch
            for c in range(nch):
                nc.scalar.activation(out=r[:, c * n:(c + 1) * n],
                                     in_=st_tiles[c],
                                     func=AF.Sqrt, bias=eps_t[:],
                                     scale=1.0 / nfeat)
            nc.vector.reciprocal(r, r)
            rr = mktile(pool, [128, T], F32, f"rr_{tag}")
            nc.gpsimd.partition_broadcast(rr, r)
            return r, rr

        # ---------------- phase A/C: q path first ----------------
        # (per-token rms scales commute through the matmuls: fold them into
        # the psum-consume multiplies instead of materializing normed x)
        qnope = [None] * 16
        qrope = [None] * 8

        with tc.tile_pool(name="pC", bufs=2) as pc_, \
             tc.tile_pool(name="pClat", bufs=1) as pcl:
            xqb = []
            for k in range(16):
                t = mktile(pcl, [128, TQ], BF16, f"xqb{k}")
                nc.scalar.dma_start(out=t[:],
                                    in_=d["xqB"][k * 128:(k + 1) * 128, :])
                xqb.append(t)
            # xq rms stats (squares of raw bf16 x; scale folded into q_a).
            # Alternate squares between ACT and DVE so the rqr chain, which
            # gates q_a's first psum consumes, completes ~2x sooner.
            stq = mktile(pst, [1, TQ], F32, "st")
            for k in range(16):
                sqt = mktile(pc_, [128, TQ], BF16, "sqq")
                if k % 2 == 0:
                    nc.scalar.activation(out=sqt, in_=xqb[k], func=AF.Square)
                else:
                    nc.vector.tensor_mul(sqt, xqb[k], xqb[k])
                nc.tensor.matmul(stq, ones_b, sqt,
                                 start=(k == 0), stop=(k == 15))
            _, rqr = rms_finish(pcl, [stq], TQ, H, "q")

            qlat_f = [mktile(pcl, [128, TQ], BF16, f"qlat{m}") for m in range(12)]
            stql = mktile(pst, [1, TQ], F32, "st")

            def qa_consume(m, c, ps):
                nc.vector.tensor_mul(qlat_f[m], ps, rqr)
                sqt = mktile(pc_, [128, TQ], BF16, "sqc")
                nc.scalar.activation(out=sqt, in_=qlat_f[m], func=AF.Square)
                nc.tensor.matmul(stql, ones_b, sqt,
                                 start=(m == 0), stop=(m == 11))

            proj(d["w_qa"], 16, 12, xqb, TQ, qa_consume, bm=4,
                 first_small=True)

            cq2 = mktile(pq, [128, TQ], F32, "cq2")
            nc.scalar.dma_start(out=cq2[0:64, :], in_=d["cos_qT"][:])
            nc.scalar.dma_start(out=cq2[64:128, :], in_=d["cos_qT"][:])
            sq2 = mktile(pq, [128, TQ], F32, "sq2")
            nc.scalar.dma_start(out=sq2[0:64, :], in_=d["sin_qT"][:])
            nc.scalar.dma_start(out=sq2[64:128, :], in_=d["sin_qT"][:])

            # xk rms stats (overlaps q_a on ACT)
            for k in range(16):
                nc.scalar.dma_start(out=xkb[k][:],
                                    in_=d["xkB"][k * 128:(k + 1) * 128, :])
            with tc.tile_pool(name="pAk", bufs=2) as pak:
                stk = [mktile(pst, [1, 512], F32, "st") for _ in range(2)]
                for k in range(16):
                    for c in range(2):
                        sqt = mktile(pak, [128, 512], BF16, "sqt")
                        nc.scalar.activation(
                            out=sqt, in_=xkb[k][:, c * 512:(c + 1) * 512],
                            func=AF.Square)
                        nc.tensor.matmul(stk[c], ones_b, sqt,
                                         start=(k == 0), stop=(k == 15))
                _, rkr = rms_finish(pkv_r, stk, TK, H, "k")

            # ---------------- kv_a + latent norm + k_pe rope ------------
            kpe_rot = mktile(pkv, [128, TK], BF16, "kpe_rot")
            with tc.tile_pool(name="pB", bufs=2) as pb, \
                 tc.tile_pool(name="pBlat", bufs=1) as pbl:
                ck_t = mktile(pbl, [ROPE, TK], F32, "ck_t")
                nc.scalar.dma_start(out=ck_t[:], in_=d["cos_kT"][:])
                sk_t = mktile(pbl, [ROPE, TK], F32, "sk_t")
                nc.scalar.dma_start(out=sk_t[:], in_=d["sin_kT"][:])
                kvlat_f = [mktile(pkv, [128, TK], BF16, f"kvlat{m}")
                           for m in range(4)]
                kpe_sb = mktile(pbl, [128, TK], F32, "kpe_sb")
                stl = [mktile(pst, [1, 512], F32, "st") for _ in range(2)]

                def kva_consume(m, c, ps):
                    sl = slice(c * 512, (c + 1) * 512)
                    if m < 4:
                        nc.vector.tensor_mul(kvlat_f[m][:, sl], ps, rkr[:, sl])
                        sqt = mktile(pb, [128, 512], BF16, "sqb")
                        nc.scalar.activation(out=sqt, in_=kvlat_f[m][:, sl],
                                             func=AF.Square)
                        nc.tensor.matmul(stl[c], ones_b, sqt,
                                         start=(m == 0), stop=(m == 3))
                    else:
                        nc.vector.tensor_mul(kpe_sb[:, sl], ps, rkr[:, sl])

                proj(d["w_kva"][:, 512:640], 16, 1, xkb, TK,
                     lambda m, c, ps: kva_consume(4, c, ps), bm=1)
                proj(d["w_kva"][:, 0:512], 16, 4, xkb, TK, kva_consume, bm=2)

                kpes = mktile(pbl, [64, TK], F32, "kpes")
                nc.sync.dma_start(out=kpes[:], in_=kpe_sb[64:128, :])
                nc.vector.tensor_mul(kpe_sb[0:64, :], kpe_sb[0:64, :], ck_t)
                nc.vector.tensor_mul(kpes, kpes, sk_t)
                nc.vector.tensor_add(kpe_rot[0:64, :], kpe_sb[0:64, :], kpes)
                nc.sync.dma_start(out=kpe_rot[64:128, :], in_=kpe_rot[0:64, :])

                rl_row, rlr = rms_finish(pkv_r, stl, TK, KVLR, "lat")
                # normed kv latent for the v-path lhsT (k_nope path applies
                # rlr at consume instead, where tk is on the free axis)
                kvlat_n = []
                for m in range(4):
                    t_ = mktile(pkv, [128, TK], BF16, f"kvlatn{m}")
                    nc.vector.tensor_mul(t_, kvlat_f[m], rlr)
                    kvlat_n.append(t_)

            # ---------------- q_b (rql folded into consumes) -------------
            _, rql = rms_finish(pcl, [stql], TQ, QLR, "ql")
            cq2q = mktile(pcl, [128, TQ], F32, "cq2q")
            nc.vector.tensor_mul(cq2q, cq2, rql)
            sq2q = mktile(pcl, [128, TQ], F32, "sq2q")
            nc.vector.tensor_mul(sq2q, sq2, rql)

            qpe_f = [mktile(pcl, [128, TQ], F32, f"qpe{j}") for j in range(8)]

            def qb_consume(m, c, ps):
                if m < 16:
                    qnope[m] = mktile(pq, [128, TQ], BF16, f"qnope{m}")
                    nc.vector.tensor_mul(qnope[m], ps, rql)
                elif m < 24:
                    nc.scalar.activation(out=qpe_f[m - 16], in_=ps, func=AF.Copy)
                else:
                    j = m - 24
                    t1 = mktile(pc_, [128, TQ], F32, "qb1")
                    nc.vector.tensor_mul(t1, qpe_f[j], cq2q)
                    t2 = mktile(pc_, [128, TQ], F32, "qb2")
                    nc.vector.tensor_mul(t2, ps, sq2q)
                    qrope[j] = mktile(pq, [128, TQ], BF16, f"qrope{j}")
                    nc.vector.tensor_add(qrope[j], t1, t2)

            proj(d["w_qb"], 12, 32, qlat_f, TQ, qb_consume, bm=4)

        # ---------------- phase D: attention ----------------
        maskt = []
        for t_ in range(8):
            mt = mktile(pq, [128, TQ], F32, f"mask{t_}")
            nc.scalar.dma_start(out=mt[:],
                                in_=d["maskT"][t_ * 128:(t_ + 1) * 128, :])
            maskt.append(mt)
        attn_out = [None] * 16
        with tc.tile_pool(name="pD", bufs=2) as pd_:
            # emit every head-pair's kv_b column loads upfront (tag slots
            # bufs=3 -> runtime pipelines 3 pairs ahead); k-tile t of a
            # (512, 256) slice lands at cols [t*256:(t+1)*256]
            kvb_tiles = []
            for hp in range(8):
                kvbn_b = pd_.tile([128, 1024], BF16, tag="kvbn", name="kvbn",
                                  bufs=3)
                nc.scalar.dma_start(
                    out=kvbn_b.rearrange("p (t m) -> p t m", t=4),
                    in_=d["w_kvb"][:, hp * 256:(hp + 1) * 256]
                    .rearrange("(t p) m -> p t m", p=128))
                kvbv_b = pd_.tile([128, 1024], BF16, tag="kvbv", name="kvbv",
                                  bufs=3)
                nc.scalar.dma_start(
                    out=kvbv_b.rearrange("p (t m) -> p t m", t=4),
                    in_=d["w_kvb"][:, 2048 + hp * 256:2048 + (hp + 1) * 256]
                    .rearrange("(t p) m -> p t m", p=128))
                kvb_tiles.append((kvbn_b, kvbv_b))

            for hp in range(8):
                kvbn_b, kvbv_b = kvb_tiles[hp]
                kvbn = [kvbn_b[:, k * 256:(k + 1) * 256] for k in range(4)]
                kvbv = [kvbv_b[:, k * 256:(k + 1) * 256] for k in range(4)]

                # k_nope MMs of the even head first: they depend only on
                # kvb + raw kv latents, not the rlr norm chain
                kn_pair = {}
                h0 = 2 * hp
                kn_pair[h0] = mktile(pd_, [128, TK], BF16, "knope")
                for c in range(2):
                    knp = mktile(pmm, [128, 512], F32, "mm")
                    for k in range(4):
                        nc.tensor.matmul(
                            knp,
                            kvbn[k][:, 0:128],
                            kvlat_f[k][:, c * 512:(c + 1) * 512],
                            start=(k == 0), stop=(k == 3))
                    nc.vector.tensor_mul(kn_pair[h0][:, c * 512:(c + 1) * 512],
                                         knp, rlr[:, c * 512:(c + 1) * 512])

                # v for the head pair, token-major [tk, 2*VD]
                v2 = []
                for tkt in range(8):
                    vp = mktile(pmm, [128, 256], F32, "mm")
                    for k in range(4):
                        nc.tensor.matmul(
                            vp,
                            kvlat_n[k][:, tkt * 128:(tkt + 1) * 128],
                            kvbv[k],
                            start=(k == 0), stop=(k == 3))
                    vt = mktile(pd_, [128, 256], BF16, f"v2_{tkt}")
                    nc.vector.tensor_copy(out=vt, in_=vp)
                    v2.append(vt)

                for h in (2 * hp, 2 * hp + 1):
                    if h in kn_pair:
                        kn = kn_pair[h]
                    else:
                        kn = mktile(pd_, [128, TK], BF16, "knope")
                        for c in range(2):
                            knp = mktile(pmm, [128, 512], F32, "mm")
                            for k in range(4):
                                nc.tensor.matmul(
                                    knp,
                                    kvbn[k][:, (h % 2) * 128:(h % 2) * 128 + 128],
                                    kvlat_f[k][:, c * 512:(c + 1) * 512],
                                    start=(k == 0), stop=(k == 3))
                            nc.vector.tensor_mul(
                                kn[:, c * 512:(c + 1) * 512],
                                knp, rlr[:, c * 512:(c + 1) * 512])

                    qr = qrope[h // 2][(h % 2) * 64:(h % 2) * 64 + 64, :]
                    p0 = (h % 2) * 64
                    ets = []
                    for tkt in range(8):
                        sps = mktile(pmm, [128, TQ], F32, "mm")
                        nc.tensor.matmul(sps, kn[:, tkt * 128:(tkt + 1) * 128],
                                         qnope[h], start=True, stop=False)
                        nc.tensor.matmul(sps,
                                         kpe_rot[p0:p0 + 64,
                                                 tkt * 128:(tkt + 1) * 128],
                                         qr, start=False, stop=True)
                        tm = mktile(pd_, [128, TQ], F32, "etmp")
                        nc.vector.tensor_add(tm, sps, maskt[tkt])
                        et = mktile(pd_, [128, TQ], BF16, f"eh{tkt}")
                        nc.scalar.activation(out=et, in_=tm, func=AF.Exp)
                        ets.append(et)
                    zps = mktile(pst, [1, TQ], F32, "st")
                    aps = mktile(pmm, [128, TQ], F32, "mm")
                    for tkt in range(8):
                        nc.tensor.matmul(zps, ones_b, ets[tkt],
                                         start=(tkt == 0), stop=(tkt == 7))
                        nc.tensor.matmul(aps,
                                         v2[tkt][:, (h % 2) * 128:(h % 2) * 128 + 128],
                                         ets[tkt],
                                         start=(tkt == 0), stop=(tkt == 7))
                    zsb = mktile(pd_, [1, TQ], F32, "zsb")
                    nc.scalar.activation(out=zsb, in_=zps, func=AF.Copy)
                    nc.vector.reciprocal(zsb, zsb)
                    rzr = mktile(pd_, [128, TQ], F32, "rzr")
                    nc.gpsimd.partition_broadcast(rzr, zsb)
                    attn_out[h] = mktile(pattn, [128, TQ], BF16, f"attn{h}")
                    nc.vector.tensor_mul(attn_out[h], aps, rzr)

        # ---------------- phase E: o_proj + residual + post-ln ----------
        h1 = [None] * 16
        for k in range(16):
            nc.scalar.dma_start(out=xqf[k][:],
                                in_=d["xqT"][k * 128:(k + 1) * 128, :])
        with tc.tile_pool(name="pE", bufs=2) as pe_:
            sto = mktile(pst, [1, TQ], F32, "st")

            def o_consume(m, c, ps):
                h1[m] = mktile(ph1, [128, TQ], F32, f"h1_{m}")
                nc.vector.tensor_add(h1[m], ps, xqf[m])
                sqt = mktile(pe_, [128, TQ], BF16, "sqe")
                nc.scalar.activation(out=sqt, in_=h1[m], func=AF.Square)
                nc.tensor.matmul(sto, ones_b, sqt,
                                 start=(m == 0), stop=(m == 15))

            proj(d["w_o"], 16, 16, attn_out, TQ, o_consume, bm=4)

            _, rmr = rms_finish(pe_, [sto], TQ, H, "m")
            h1n = []
            for m in range(16):
                t = mktile(ph1, [128, TQ], BF16, f"h1n{m}")
                nc.vector.tensor_mul(t, h1[m], rmr)
                h1n.append(t)

        # ---------------- phase F: MLP ----------------
        with tc.tile_pool(name="pF", bufs=1) as pf, \
             tc.tile_pool(name="pFt", bufs=2) as pft:
            y = [mktile(pf, [128, TQ], BF16, f"y{m}") for m in range(64)]

            def gate_consume(m, c, ps):
                # silu(x) = x * sigmoid(x) (CoreSim has no Silu)
                sg = mktile(pft, [128, TQ], F32, "sg")
                nc.scalar.activation(out=sg, in_=ps, func=AF.Sigmoid)
                nc.vector.tensor_mul(y[m], ps, sg)

            def up_consume(m, c, ps):
                nc.vector.tensor_mul(y[m], ps, y[m])

            proj(d["w_gate"], 16, 64, h1n, TQ, gate_consume, bm=4)
            proj(d["w_up"], 16, 64, h1n, TQ, up_consume, bm=4)

            def down_consume(m, c, ps):
                ot = mktile(pft, [128, TQ], F32, "outt")
                nc.vector.tensor_add(ot, ps, h1[m])
                nc.sync.dma_start(out=out_d[m * 128:(m + 1) * 128, :], in_=ot[:])

            proj(d["w_down"], 64, 16, y, TQ, down_consume, bm=4)

    nc.compile()
    return nc


# ---------------------------------------------------------------- host -----

def _prep_weights(inputs):
    w = {}
    deint = np.concatenate([np.arange(0, ROPE, 2), np.arange(1, ROPE, 2)])
    swap = np.concatenate([np.arange(32, 64), np.arange(0, 32)])

    in_ln = np.asarray(inputs['in_ln_w'], np.float32)
    w['w_qa'] = np.ascontiguousarray(
        (np.asarray(inputs['q_a_w'], np.float32) * in_ln[:, None]).astype(bf16))
    qb = (np.asarray(inputs['q_b_w'], np.float32)
          * np.asarray(inputs['q_a_ln_w'], np.float32)[:, None] * SCALE
          ).reshape(QLR, NH, QHD)
    qb_nope = qb[:, :, :NOPE].reshape(QLR, NH * NOPE)
    qb_rope = qb[:, :, NOPE:][:, :, deint]
    w['w_qb'] = np.ascontiguousarray(np.concatenate(
        [qb_nope, qb_rope.reshape(QLR, NH * ROPE),
         qb_rope[:, :, swap].reshape(QLR, NH * ROPE)], axis=1).astype(bf16))
    kva = np.asarray(inputs['kv_a_w'], np.float32) * in_ln[:, None]
    kva_pe = kva[:, KVLR:][:, deint]
    w['w_kva'] = np.ascontiguousarray(np.concatenate(
        [kva[:, :KVLR], kva_pe, kva_pe[:, swap]], axis=1).astype(bf16))
    kvb = (np.asarray(inputs['kv_b_w'], np.float32)
           * np.asarray(inputs['kv_a_ln_w'], np.float32)[:, None]
           ).reshape(KVLR, NH, NOPE + VD)
    w['w_kvb'] = np.ascontiguousarray(np.concatenate(
        [kvb[:, :, :NOPE].reshape(KVLR, NH * NOPE),
         kvb[:, :, NOPE:].reshape(KVLR, NH * VD)], axis=1).astype(bf16))
    w['w_o'] = np.ascontiguousarray(np.asarray(inputs['o_w'], np.float32).astype(bf16))
    post_ln = np.asarray(inputs['post_ln_w'], np.float32)
    w['w_gate'] = np.ascontiguousarray(
        (np.asarray(inputs['gate_w'], np.float32) * post_ln[:, None]).astype(bf16))
    w['w_up'] = np.ascontiguousarray(
        (np.asarray(inputs['up_w'], np.float32) * post_ln[:, None]).astype(bf16))
    w['w_down'] = np.ascontiguousarray(np.asarray(inputs['down_w'], np.float32).astype(bf16))
    return w


def _prep_core(inputs, core):
    b, c = core // 4, core % 4
    rows = slice(c * TQ, (c + 1) * TQ)
    dd = {}
    hid = np.asarray(inputs['hidden_states'][b], np.float32)
    hidT = np.ascontiguousarray(hid.T)
    dd['xkB'] = hidT.astype(bf16)
    dd['xqB'] = np.ascontiguousarray(hidT[:, rows]).astype(bf16)
    dd['xqT'] = np.ascontiguousarray(hidT[:, rows])
    pos = np.asarray(inputs['position_ids'][b]).astype(np.int64)
    cos = np.asarray(inputs['cos'], np.float32)[pos]
    sin = np.asarray(inputs['sin'], np.float32)[pos]
    sgn = np.concatenate([-np.ones(32, np.float32), np.ones(32, np.float32)])
    dd['cos_kT'] = np.ascontiguousarray(cos.T)
    dd['sin_kT'] = np.ascontiguousarray((sin * sgn[None, :]).T)
    dd['cos_qT'] = np.ascontiguousarray(cos[rows].T)
    dd['sin_qT'] = np.ascontiguousarray((sin[rows] * sgn[None, :]).T)
    q_pos = np.arange(c * TQ, (c + 1) * TQ)
    k_pos = np.arange(S)
    vis = (k_pos[:, None] <= q_pos[None, :]) \
        & (np.asarray(inputs['attention_mask'][b]) > 0)[:, None]
    dd['maskT'] = np.where(vis, 0.0, -1e30).astype(np.float32)
    return dd


def prep_in_maps(inputs):
    w = _prep_weights(inputs)
    in_maps = []
    for core in range(N_CORES):
        m = dict(w)
        m.update(_prep_core(inputs, core))
        in_maps.append(m)
    return in_maps


_NC = None


def _get_nc():
    global _NC
    if _NC is None:
        _NC = build_nc()
    return _NC


_EXEC = None   # (jitted_fn, in_names, out_names, out_avals, mesh)


def _get_exec():
    """Build the 8-core sharded executable once (mirrors
    bass2jax.run_bass_via_pjrt's multi-core path, without donation so the
    callable can be re-invoked for timing)."""
    global _EXEC
    if _EXEC is None:
        import jax
        from jax.sharding import Mesh, PartitionSpec
        from jax.experimental.shard_map import shard_map
        import concourse.mybir as mybir_
        from concourse import bass2jax

        nc = _get_nc()
        bass2jax.install_neuronx_cc_hook()
        pname = nc.partition_id_tensor.name if nc.partition_id_tensor else None
        in_names, out_names, out_avals = [], [], []
        for alloc in nc.m.functions[0].allocations:
            if not isinstance(alloc, mybir_.MemoryLocationSet):
                continue
            name = alloc.memorylocations[0].name
            if alloc.kind == "ExternalInput":
                if name != pname:
                    in_names.append(name)
            elif alloc.kind == "ExternalOutput":
                out_names.append(name)
                out_avals.append(jax.core.ShapedArray(
                    tuple(alloc.tensor_shape), mybir_.dt.np(alloc.dtype)))
        n_params = len(in_names)
        all_names = in_names + out_names
        if pname is not None:
            all_names = all_names + [pname]

        def _body(*args):
            operands = list(args)
            if pname is not None:
                operands.append(bass2jax.partition_id_tensor())
            outs = bass2jax._bass_exec_p.bind(
                *operands,
                out_avals=tuple(out_avals),
                in_names=tuple(all_names),
                out_names=tuple(out_names),
                lowering_input_output_aliases=(),
                sim_require_finite=True,
                sim_require_nnan=True,
                nc=nc,
            )
            return tuple(outs)

        devices = jax.devices()[:N_CORES]
        mesh = Mesh(np.asarray(devices), ("core",))
        nin = n_params + len(out_names)
        fn = jax.jit(shard_map(
            _body, mesh=mesh,
            in_specs=(PartitionSpec("core"),) * nin,
            out_specs=(PartitionSpec("core"),) * len(out_names),
            check_rep=False))
        _EXEC = (fn, in_names, out_names, out_avals, mesh)
    return _EXEC


def device_args(inputs):
    """Concatenated (and device-put) arg list for the sharded executable."""
    import jax
    from jax.sharding import NamedSharding, PartitionSpec

    fn, in_names, out_names, out_avals, mesh = _get_exec()
    in_maps = prep_in_maps(inputs)
    args = [np.concatenate([in_maps[c][n] for c in range(N_CORES)], axis=0)
            for n in in_names]
    for av in out_avals:
        args.append(np.zeros((N_CORES * av.shape[0],) + av.shape[1:], av.dtype))
    sh = NamedSharding(mesh, PartitionSpec("core"))
    return [jax.device_put(a, sh) for a in args]


def run(inputs):
    import jax

    fn, in_names, out_names, out_avals, mesh = _get_exec()
    args = device_args(inputs)
    outs = jax.block_until_ready(fn(*args))
    out_full = np.asarray(outs[0]).reshape(N_CORES, H, TQ)
    out = np.zeros((B, S, H), np.float32)
    for core in range(N_CORES):
        b, c = core // 4, core % 4
        out[b, c * TQ:(c + 1) * TQ] = out_full[core].T
    return out


def kernel(**inputs):
    return run(inputs)



# revision 12
# speedup vs baseline: 69.0945x; 69.0945x over previous
"""DeepseekV3 decoder layer on 8 TRN2 NeuronCores.

Sharding: pure data parallel over tokens, zero collectives. B=2, S=1024 ->
2048 tokens; core = (batch b, quarter c) owns 256 query tokens. Each core
recomputes the full-batch KV path (~+10% FLOPs) so attention needs no
cross-core traffic; host assembles the 8 (2048, 256) output slices.

Device kernel: feature-major activations (feat on partitions, tokens on the
free dim) for every matmul; bf16 weights/operands with f32 PSUM accumulation;
RMS-norm partition reductions via Square + ones-matmul (float32r); rope via
host-side column permutation (deinterleave) + duplicated swapped columns so
rotate_half needs no cross-partition moves; softmax without max subtraction
(scores are O(1) by construction); scores computed transposed (tk, tq) so
attn@v contracts tk on partitions for both operands.
"""
import numpy as np
import ml_dtypes

import concourse.bass as bass
import concourse.mybir as mybir
import concourse.tile as tile
from concourse import bacc
from concourse import bass_utils

F32 = mybir.dt.float32
BF16 = mybir.dt.bfloat16
F32R = mybir.dt.float32r
AF = mybir.ActivationFunctionType

H, NH, QLR, KVLR = 2048, 16, 1536, 512
NOPE, ROPE, VD = 128, 64, 128
QHD = NOPE + ROPE
I, B, S = 8192, 2, 1024
EPS = 1e-6
SCALE = QHD ** -0.5
N_CORES = 8
TQ = 256   # query tokens per core
TK = 1024  # key tokens (full batch) per core

bf16 = ml_dtypes.bfloat16


# ---------------------------------------------------------------- device ---

def build_nc(loop_n=1):
    """loop_n > 1 wraps the whole layer in a device-side For_i loop (single
    code copy, runtime trip count) so per-layer HW time can be measured with
    the fixed per-call dispatch latency amortized away. The computation per
    iteration is identical to loop_n=1."""
    from contextlib import ExitStack

    nc = bacc.Bacc("TRN2", target_bir_lowering=False, debug=False)

    d = {}

    def din(name, shape, dt=F32):
        d[name] = nc.dram_tensor(name, shape, dt, kind="ExternalInput").ap()

    din("xkB", (H, TK), BF16)           # raw hidden^T (full batch), bf16
    din("xqB", (H, TQ), BF16)           # raw hidden^T (query slice), bf16
    din("xqT", (H, TQ))                 # residual, f32
    din("cos_kT", (ROPE, TK))
    din("sin_kT", (ROPE, TK))
    din("cos_qT", (ROPE, TQ))
    din("sin_qT", (ROPE, TQ))
    din("maskT", (TK, TQ))
    din("w_qa", (H, QLR), BF16)
    din("w_qb", (QLR, 4096), BF16)      # [nope 16x128 | rope 16x64 | rope_swap 16x64]
    din("w_kva", (H, 640), BF16)        # [lat 512 | pe 64 | pe_swap 64]
    din("w_kvb", (KVLR, 4096), BF16)    # [k_nope 16x128 | v 16x128]
    din("w_o", (H, H), BF16)
    din("w_gate", (H, I), BF16)
    din("w_up", (H, I), BF16)
    din("w_down", (I, H), BF16)
    out_d = nc.dram_tensor("out", (H, TQ), F32, kind="ExternalOutput").ap()

    with tile.TileContext(nc) as tc, ExitStack() as ctx:
        pl0 = ctx.enter_context(tc.tile_pool(name="pl0", bufs=1))
        pw = ctx.enter_context(tc.tile_pool(name="wslab", bufs=3))
        ph1 = ctx.enter_context(tc.tile_pool(name="ph1", bufs=1))      # h1/h1n (E..F)
        pxqf = ctx.enter_context(tc.tile_pool(name="pxqf", bufs=1))    # xqf f32 (0..E)
        pattn = ctx.enter_context(tc.tile_pool(name="pattn", bufs=1))  # attn_out (D..E)
        pkv = ctx.enter_context(tc.tile_pool(name="pkv", bufs=1))      # kv products (B..D)
        pq = ctx.enter_context(tc.tile_pool(name="pq", bufs=1))        # q products (C..D)
        pkv_r = pkv  # r-vectors resident alongside kv products
        pxb = ctx.enter_context(tc.tile_pool(name="pxb", bufs=1))      # xkB/xqB resident
        pmm = ctx.enter_context(tc.tile_pool(name="pmm", bufs=6, space="PSUM"))
        pst = ctx.enter_context(tc.tile_pool(name="pst", bufs=2, space="PSUM"))

        def mktile(pool, shape, dtype, tag):
            return pool.tile(shape, dtype, tag=tag, name=tag)

        ones_b = mktile(pl0, [128, 1], BF16, "ones_b")
        nc.vector.memset(ones_b, 1.0)
        ones_f = mktile(pl0, [128, 1], F32, "ones_f")
        nc.vector.memset(ones_f, 1.0)
        eps_t = mktile(pl0, [1, 1], F32, "eps_t")
        nc.vector.memset(eps_t, EPS)

        # raw activations, bf16, feature-major (resident). Bulk input loads
        # ride the Activation engine's HWDGE queue so weight-slab DMAs (SP
        # queue) are not stuck behind them.
        xkb = [mktile(pxb, [128, TK], BF16, f"xkb{k}") for k in range(16)]
        xqf = [mktile(pxqf, [128, TQ], F32, f"xqf{k}") for k in range(16)]

        # ---------------- generic streamed projection ----------------
        def proj(w_ap, Kt, Mt, rhs_tiles, T, consume, bm=4, kg=4,
                 first_small=False):
            """psum[m, c] = sum_k W[k,m-slice].T @ rhs[k][:, c-slice].

            One psum bank per (m, c) unit (a 2KB psum zero-region admits only
            one pending accumulation group). Weight DMAs fetch kg k-tiles per
            transfer via a 3D access pattern to amortize the ~625ns HWDGE
            fixed cost per dma_start.
            """
            nchunk = max(1, T // 512)
            N = T // nchunk
            for m0 in range(0, Mt, bm):
                ms = list(range(m0, min(m0 + bm, Mt)))
                bw = len(ms) * 128
                units = [(m, c) for m in ms for c in range(nchunk)]
                psap = {}
                for (m, c) in units:
                    psap[(m, c)] = mktile(pmm, [128, N], F32, "mm")
                if first_small and m0 == 0:
                    # smaller leading k-groups so the first slab lands fast
                    groups = [(0, 1), (1, 1)]
                    k0_ = 2
                    while k0_ < Kt:
                        nk_ = min(kg, Kt - k0_)
                        groups.append((k0_, nk_))
                        k0_ += nk_
                else:
                    groups = [(k0_, min(kg, Kt - k0_))
                              for k0_ in range(0, Kt, kg)]
                for k0, nk in groups:
                    wsl = pw.tile([128, nk * bw], BF16, tag="wsl", name="wsl")
                    src = w_ap[k0 * 128:(k0 + nk) * 128,
                               m0 * 128:m0 * 128 + bw]
                    nc.sync.dma_start(
                        out=wsl.rearrange("p (t m) -> p t m", t=nk),
                        in_=src.rearrange("(t p) m -> p t m", p=128))
                    for dk in range(nk):
                        k = k0 + dk
                        st = (k == 0)
                        sp = (k == Kt - 1)
                        for mi, m in enumerate(ms):
                            for c in range(nchunk):
                                nc.tensor.matmul(
                                    psap[(m, c)],
                                    wsl[:, (dk * len(ms) + mi) * 128:
                                        (dk * len(ms) + mi + 1) * 128],
                                    rhs_tiles[k][:, c * N:(c + 1) * N],
                                    start=st, stop=sp)
                for (m, c) in units:
                    consume(m, c, psap[(m, c)])

        def rms_finish(pool, st_tiles, T, nfeat, tag):
            """r = 1/sqrt(sumsq/nfeat + eps): returns ([1,T] row, [128,T]
            partition-replicated)."""
            r = mktile(pool, [1, T], F32, f"r_{tag}")
            nch = len(st_tiles)
            n = T // nch
            for c in range(nch):
                nc.scalar.activation(out=r[:, c * n:(c + 1) * n],
                                     in_=st_tiles[c],
                                     func=AF.Sqrt, bias=eps_t[:],
                                     scale=1.0 / nfeat)
            nc.vector.reciprocal(r, r)
            rr = mktile(pool, [128, T], F32, f"rr_{tag}")
            nc.gpsimd.partition_broadcast(rr, r)
            return r, rr

        if loop_n > 1:
            ctx.enter_context(tc.For_i(
                0, loop_n, 1,
                hint_engines=(mybir.EngineType.PE, mybir.EngineType.DVE,
                              mybir.EngineType.Activation, mybir.EngineType.SP,
                              mybir.EngineType.Pool)))

        # ---------------- phase A/C: q path first ----------------
        # (per-token rms scales commute through the matmuls: fold them into
        # the psum-consume multiplies instead of materializing normed x)
        qnope = [None] * 16
        qrope = [None] * 8

        with tc.tile_pool(name="pC", bufs=2) as pc_, \
             tc.tile_pool(name="pClat", bufs=1) as pcl:
            xqb = []
            for k in range(16):
                t = mktile(pcl, [128, TQ], BF16, f"xqb{k}")
                nc.scalar.dma_start(out=t[:],
                                    in_=d["xqB"][k * 128:(k + 1) * 128, :])
                xqb.append(t)
            # xq rms stats (squares of raw bf16 x; scale folded into q_a).
            # Alternate squares between ACT and DVE so the rqr chain, which
            # gates q_a's first psum consumes, completes ~2x sooner.
            stq = mktile(pst, [1, TQ], F32, "st")
            for k in range(16):
                sqt = mktile(pc_, [128, TQ], BF16, "sqq")
                if k % 2 == 0:
                    nc.scalar.activation(out=sqt, in_=xqb[k], func=AF.Square)
                else:
                    nc.vector.tensor_mul(sqt, xqb[k], xqb[k])
                nc.tensor.matmul(stq, ones_b, sqt,
                                 start=(k == 0), stop=(k == 15))
            _, rqr = rms_finish(pcl, [stq], TQ, H, "q")

            qlat_f = [mktile(pcl, [128, TQ], BF16, f"qlat{m}") for m in range(12)]
            stql = mktile(pst, [1, TQ], F32, "st")

            def qa_consume(m, c, ps):
                nc.vector.tensor_mul(qlat_f[m], ps, rqr)
                sqt = mktile(pc_, [128, TQ], BF16, "sqc")
                nc.scalar.activation(out=sqt, in_=qlat_f[m], func=AF.Square)
                nc.tensor.matmul(stql, ones_b, sqt,
                                 start=(m == 0), stop=(m == 11))

            proj(d["w_qa"], 16, 12, xqb, TQ, qa_consume, bm=4,
                 first_small=True)

            cq2 = mktile(pq, [128, TQ], F32, "cq2")
            nc.scalar.dma_start(out=cq2[0:64, :], in_=d["cos_qT"][:])
            nc.scalar.dma_start(out=cq2[64:128, :], in_=d["cos_qT"][:])
            sq2 = mktile(pq, [128, TQ], F32, "sq2")
            nc.scalar.dma_start(out=sq2[0:64, :], in_=d["sin_qT"][:])
            nc.scalar.dma_start(out=sq2[64:128, :], in_=d["sin_qT"][:])

            # xk rms stats (overlaps q_a on ACT)
            for k in range(16):
                nc.scalar.dma_start(out=xkb[k][:],
                                    in_=d["xkB"][k * 128:(k + 1) * 128, :])
            with tc.tile_pool(name="pAk", bufs=2) as pak:
                stk = [mktile(pst, [1, 512], F32, "st") for _ in range(2)]
                for k in range(16):
                    for c in range(2):
                        sqt = mktile(pak, [128, 512], BF16, "sqt")
                        nc.scalar.activation(
                            out=sqt, in_=xkb[k][:, c * 512:(c + 1) * 512],
                            func=AF.Square)
                        nc.tensor.matmul(stk[c], ones_b, sqt,
                                         start=(k == 0), stop=(k == 15))
                _, rkr = rms_finish(pkv_r, stk, TK, H, "k")

            # ---------------- kv_a + latent norm + k_pe rope ------------
            kpe_rot = mktile(pkv, [128, TK], BF16, "kpe_rot")
            with tc.tile_pool(name="pB", bufs=2) as pb, \
                 tc.tile_pool(name="pBlat", bufs=1) as pbl:
                ck_t = mktile(pbl, [ROPE, TK], F32, "ck_t")
                nc.scalar.dma_start(out=ck_t[:], in_=d["cos_kT"][:])
                sk_t = mktile(pbl, [ROPE, TK], F32, "sk_t")
                nc.scalar.dma_start(out=sk_t[:], in_=d["sin_kT"][:])
                kvlat_f = [mktile(pkv, [128, TK], BF16, f"kvlat{m}")
                           for m in range(4)]
                kpe_sb = mktile(pbl, [128, TK], F32, "kpe_sb")
                stl = [mktile(pst, [1, 512], F32, "st") for _ in range(2)]

                def kva_consume(m, c, ps):
                    sl = slice(c * 512, (c + 1) * 512)
                    if m < 4:
                        nc.vector.tensor_mul(kvlat_f[m][:, sl], ps, rkr[:, sl])
                        sqt = mktile(pb, [128, 512], BF16, "sqb")
                        nc.scalar.activation(out=sqt, in_=kvlat_f[m][:, sl],
                                             func=AF.Square)
                        nc.tensor.matmul(stl[c], ones_b, sqt,
                                         start=(m == 0), stop=(m == 3))
                    else:
                        nc.vector.tensor_mul(kpe_sb[:, sl], ps, rkr[:, sl])

                proj(d["w_kva"][:, 512:640], 16, 1, xkb, TK,
                     lambda m, c, ps: kva_consume(4, c, ps), bm=1)
                proj(d["w_kva"][:, 0:512], 16, 4, xkb, TK, kva_consume, bm=2)

                kpes = mktile(pbl, [64, TK], F32, "kpes")
                nc.sync.dma_start(out=kpes[:], in_=kpe_sb[64:128, :])
                nc.vector.tensor_mul(kpe_sb[0:64, :], kpe_sb[0:64, :], ck_t)
                nc.vector.tensor_mul(kpes, kpes, sk_t)
                nc.vector.tensor_add(kpe_rot[0:64, :], kpe_sb[0:64, :], kpes)
                nc.sync.dma_start(out=kpe_rot[64:128, :], in_=kpe_rot[0:64, :])

                rl_row, rlr = rms_finish(pkv_r, stl, TK, KVLR, "lat")
                # normed kv latent for the v-path lhsT (k_nope path applies
                # rlr at consume instead, where tk is on the free axis)
                kvlat_n = []
                for m in range(4):
                    t_ = mktile(pkv, [128, TK], BF16, f"kvlatn{m}")
                    nc.vector.tensor_mul(t_, kvlat_f[m], rlr)
                    kvlat_n.append(t_)

            # ---------------- q_b (rql folded into consumes) -------------
            _, rql = rms_finish(pcl, [stql], TQ, QLR, "ql")
            cq2q = mktile(pcl, [128, TQ], F32, "cq2q")
            nc.vector.tensor_mul(cq2q, cq2, rql)
            sq2q = mktile(pcl, [128, TQ], F32, "sq2q")
            nc.vector.tensor_mul(sq2q, sq2, rql)

            qpe_f = [mktile(pcl, [128, TQ], F32, f"qpe{j}") for j in range(8)]

            def qb_consume(m, c, ps):
                if m < 16:
                    qnope[m] = mktile(pq, [128, TQ], BF16, f"qnope{m}")
                    nc.vector.tensor_mul(qnope[m], ps, rql)
                elif m < 24:
                    nc.scalar.activation(out=qpe_f[m - 16], in_=ps, func=AF.Copy)
                else:
                    j = m - 24
                    t1 = mktile(pc_, [128, TQ], F32, "qb1")
                    nc.vector.tensor_mul(t1, qpe_f[j], cq2q)
                    t2 = mktile(pc_, [128, TQ], F32, "qb2")
                    nc.vector.tensor_mul(t2, ps, sq2q)
                    qrope[j] = mktile(pq, [128, TQ], BF16, f"qrope{j}")
                    nc.vector.tensor_add(qrope[j], t1, t2)

            proj(d["w_qb"], 12, 32, qlat_f, TQ, qb_consume, bm=4)

        # ---------------- phase D: attention ----------------
        # Query pairing: cols 0:128 = query block c (rows c*128..), cols
        # 128:256 = block 7-c. Uniformly across cores: key tiles 0..3 are
        # scored full-width (any low block's visible set fits in them; the
        # mask kills the rest), key tiles 4..7 only for the high block.
        maskt = []
        for t_ in range(8):
            if t_ < 4:
                mt = mktile(pq, [128, TQ], F32, f"mask{t_}")
                nc.scalar.dma_start(out=mt[:],
                                    in_=d["maskT"][t_ * 128:(t_ + 1) * 128, :])
            else:
                mt = mktile(pq, [128, 128], F32, f"mask{t_}")
                nc.scalar.dma_start(out=mt[:],
                                    in_=d["maskT"][t_ * 128:(t_ + 1) * 128,
                                                   128:256])
            maskt.append(mt)
        attn_out = [None] * 16
        with tc.tile_pool(name="pD", bufs=2) as pd_:
            # emit every head-pair's kv_b column loads upfront (tag slots
            # bufs=3 -> runtime pipelines 3 pairs ahead); k-tile t of a
            # (512, 256) slice lands at cols [t*256:(t+1)*256]
            kvb_tiles = []
            for hp in range(8):
                kvbn_b = pd_.tile([128, 1024], BF16, tag="kvbn", name="kvbn",
                                  bufs=3)
                nc.scalar.dma_start(
                    out=kvbn_b.rearrange("p (t m) -> p t m", t=4),
                    in_=d["w_kvb"][:, hp * 256:(hp + 1) * 256]
                    .rearrange("(t p) m -> p t m", p=128))
                kvbv_b = pd_.tile([128, 1024], BF16, tag="kvbv", name="kvbv",
                                  bufs=3)
                nc.scalar.dma_start(
                    out=kvbv_b.rearrange("p (t m) -> p t m", t=4),
                    in_=d["w_kvb"][:, 2048 + hp * 256:2048 + (hp + 1) * 256]
                    .rearrange("(t p) m -> p t m", p=128))
                kvb_tiles.append((kvbn_b, kvbv_b))

            for hp in range(8):
                kvbn_b, kvbv_b = kvb_tiles[hp]
                kvbn = [kvbn_b[:, k * 256:(k + 1) * 256] for k in range(4)]
                kvbv = [kvbv_b[:, k * 256:(k + 1) * 256] for k in range(4)]

                # k_nope MMs of the even head first: they depend only on
                # kvb + raw kv latents, not the rlr norm chain
                kn_pair = {}
                h0 = 2 * hp
                kn_pair[h0] = mktile(pd_, [128, TK], BF16, "knope")
                for c in range(2):
                    knp = mktile(pmm, [128, 512], F32, "mm")
                    for k in range(4):
                        nc.tensor.matmul(
                            knp,
                            kvbn[k][:, 0:128],
                            kvlat_f[k][:, c * 512:(c + 1) * 512],
                            start=(k == 0), stop=(k == 3))
                    nc.vector.tensor_mul(kn_pair[h0][:, c * 512:(c + 1) * 512],
                                         knp, rlr[:, c * 512:(c + 1) * 512])

                # v for the head pair, token-major [tk, 2*VD]
                v2 = []
                for tkt in range(8):
                    vp = mktile(pmm, [128, 256], F32, "mm")
                    for k in range(4):
                        nc.tensor.matmul(
                            vp,
                            kvlat_n[k][:, tkt * 128:(tkt + 1) * 128],
                            kvbv[k],
                            start=(k == 0), stop=(k == 3))
                    vt = mktile(pd_, [128, 256], BF16, f"v2_{tkt}")
                    nc.vector.tensor_copy(out=vt, in_=vp)
                    v2.append(vt)

                for h in (2 * hp, 2 * hp + 1):
                    if h in kn_pair:
                        kn = kn_pair[h]
                    else:
                        kn = mktile(pd_, [128, TK], BF16, "knope")
                        for c in range(2):
                            knp = mktile(pmm, [128, 512], F32, "mm")
                            for k in range(4):
                                nc.tensor.matmul(
                                    knp,
                                    kvbn[k][:, (h % 2) * 128:(h % 2) * 128 + 128],
                                    kvlat_f[k][:, c * 512:(c + 1) * 512],
                                    start=(k == 0), stop=(k == 3))
                            nc.vector.tensor_mul(
                                kn[:, c * 512:(c + 1) * 512],
                                knp, rlr[:, c * 512:(c + 1) * 512])

                    qr = qrope[h // 2][(h % 2) * 64:(h % 2) * 64 + 64, :]
                    p0 = (h % 2) * 64
                    ets = []
                    for tkt in range(8):
                        cs = slice(0, TQ) if tkt < 4 else slice(128, TQ)
                        nq = cs.stop - cs.start
                        sps = mktile(pmm, [128, nq], F32, "mm")
                        nc.tensor.matmul(sps, kn[:, tkt * 128:(tkt + 1) * 128],
                                         qnope[h][:, cs], start=True, stop=False)
                        nc.tensor.matmul(sps,
                                         kpe_rot[p0:p0 + 64,
                                                 tkt * 128:(tkt + 1) * 128],
                                         qr[:, cs], start=False, stop=True)
                        tm = mktile(pd_, [128, nq], F32, "etmp")
                        nc.vector.tensor_add(tm, sps, maskt[tkt])
                        et = mktile(pd_, [128, nq], BF16, f"eh{tkt}")
                        nc.scalar.activation(out=et, in_=tm, func=AF.Exp)
                        ets.append(et)
                    zps = mktile(pst, [1, TQ], F32, "st")
                    aps = mktile(pmm, [128, TQ], F32, "mm")
                    for tkt in range(8):
                        cs = slice(0, TQ) if tkt < 4 else slice(128, TQ)
                        nc.tensor.matmul(zps[:, cs], ones_b, ets[tkt],
                                         start=(tkt == 0), stop=(tkt == 7))
                        nc.tensor.matmul(aps[:, cs],
                                         v2[tkt][:, (h % 2) * 128:(h % 2) * 128 + 128],
                                         ets[tkt],
                                         start=(tkt == 0), stop=(tkt == 7))
                    zsb = mktile(pd_, [1, TQ], F32, "zsb")
                    nc.scalar.activation(out=zsb, in_=zps, func=AF.Copy)
                    nc.vector.reciprocal(zsb, zsb)
                    rzr = mktile(pd_, [128, TQ], F32, "rzr")
                    nc.gpsimd.partition_broadcast(rzr, zsb)
                    attn_out[h] = mktile(pattn, [128, TQ], BF16, f"attn{h}")
                    nc.vector.tensor_mul(attn_out[h], aps, rzr)

        # ---------------- phase E: o_proj + residual + post-ln ----------
        h1 = [None] * 16
        for k in range(16):
            nc.scalar.dma_start(out=xqf[k][:],
                                in_=d["xqT"][k * 128:(k + 1) * 128, :])
        with tc.tile_pool(name="pE", bufs=2) as pe_:
            sto = mktile(pst, [1, TQ], F32, "st")

            def o_consume(m, c, ps):
                h1[m] = mktile(ph1, [128, TQ], F32, f"h1_{m}")
                nc.vector.tensor_add(h1[m], ps, xqf[m])
                sqt = mktile(pe_, [128, TQ], BF16, "sqe")
                nc.scalar.activation(out=sqt, in_=h1[m], func=AF.Square)
                nc.tensor.matmul(sto, ones_b, sqt,
                                 start=(m == 0), stop=(m == 15))

            proj(d["w_o"], 16, 16, attn_out, TQ, o_consume, bm=4)

            _, rmr = rms_finish(pe_, [sto], TQ, H, "m")
            h1n = []
            for m in range(16):
                t = mktile(ph1, [128, TQ], BF16, f"h1n{m}")
                nc.vector.tensor_mul(t, h1[m], rmr)
                h1n.append(t)

        # ---------------- phase F: MLP ----------------
        with tc.tile_pool(name="pF", bufs=1) as pf, \
             tc.tile_pool(name="pFt", bufs=2) as pft:
            y = [mktile(pf, [128, TQ], BF16, f"y{m}") for m in range(64)]

            def gate_consume(m, c, ps):
                # silu(x) = x * sigmoid(x) (CoreSim has no Silu)
                sg = mktile(pft, [128, TQ], F32, "sg")
                nc.scalar.activation(out=sg, in_=ps, func=AF.Sigmoid)
                nc.vector.tensor_mul(y[m], ps, sg)

            def up_consume(m, c, ps):
                nc.vector.tensor_mul(y[m], ps, y[m])

            proj(d["w_gate"], 16, 64, h1n, TQ, gate_consume, bm=4)
            proj(d["w_up"], 16, 64, h1n, TQ, up_consume, bm=4)

            def down_consume(m, c, ps):
                ot = mktile(pft, [128, TQ], F32, "outt")
                nc.vector.tensor_add(ot, ps, h1[m])
                nc.sync.dma_start(out=out_d[m * 128:(m + 1) * 128, :], in_=ot[:])

            proj(d["w_down"], 64, 16, y, TQ, down_consume, bm=4)

    nc.compile()
    return nc


# ---------------------------------------------------------------- host -----

def _prep_weights(inputs):
    w = {}
    deint = np.concatenate([np.arange(0, ROPE, 2), np.arange(1, ROPE, 2)])
    swap = np.concatenate([np.arange(32, 64), np.arange(0, 32)])

    in_ln = np.asarray(inputs['in_ln_w'], np.float32)
    w['w_qa'] = np.ascontiguousarray(
        (np.asarray(inputs['q_a_w'], np.float32) * in_ln[:, None]).astype(bf16))
    qb = (np.asarray(inputs['q_b_w'], np.float32)
          * np.asarray(inputs['q_a_ln_w'], np.float32)[:, None] * SCALE
          ).reshape(QLR, NH, QHD)
    qb_nope = qb[:, :, :NOPE].reshape(QLR, NH * NOPE)
    qb_rope = qb[:, :, NOPE:][:, :, deint]
    w['w_qb'] = np.ascontiguousarray(np.concatenate(
        [qb_nope, qb_rope.reshape(QLR, NH * ROPE),
         qb_rope[:, :, swap].reshape(QLR, NH * ROPE)], axis=1).astype(bf16))
    kva = np.asarray(inputs['kv_a_w'], np.float32) * in_ln[:, None]
    kva_pe = kva[:, KVLR:][:, deint]
    w['w_kva'] = np.ascontiguousarray(np.concatenate(
        [kva[:, :KVLR], kva_pe, kva_pe[:, swap]], axis=1).astype(bf16))
    kvb = (np.asarray(inputs['kv_b_w'], np.float32)
           * np.asarray(inputs['kv_a_ln_w'], np.float32)[:, None]
           ).reshape(KVLR, NH, NOPE + VD)
    w['w_kvb'] = np.ascontiguousarray(np.concatenate(
        [kvb[:, :, :NOPE].reshape(KVLR, NH * NOPE),
         kvb[:, :, NOPE:].reshape(KVLR, NH * VD)], axis=1).astype(bf16))
    w['w_o'] = np.ascontiguousarray(np.asarray(inputs['o_w'], np.float32).astype(bf16))
    post_ln = np.asarray(inputs['post_ln_w'], np.float32)
    w['w_gate'] = np.ascontiguousarray(
        (np.asarray(inputs['gate_w'], np.float32) * post_ln[:, None]).astype(bf16))
    w['w_up'] = np.ascontiguousarray(
        (np.asarray(inputs['up_w'], np.float32) * post_ln[:, None]).astype(bf16))
    w['w_down'] = np.ascontiguousarray(np.asarray(inputs['down_w'], np.float32).astype(bf16))
    return w


def _core_rows(c):
    """Query token rows for core quarter c: block c plus block 7-c."""
    return np.concatenate([np.arange(c * 128, (c + 1) * 128),
                           np.arange((7 - c) * 128, (8 - c) * 128)])


def _prep_core(inputs, core):
    b, c = core // 4, core % 4
    rows = _core_rows(c)
    dd = {}
    hid = np.asarray(inputs['hidden_states'][b], np.float32)
    hidT = np.ascontiguousarray(hid.T)
    dd['xkB'] = hidT.astype(bf16)
    dd['xqB'] = np.ascontiguousarray(hidT[:, rows]).astype(bf16)
    dd['xqT'] = np.ascontiguousarray(hidT[:, rows])
    pos = np.asarray(inputs['position_ids'][b]).astype(np.int64)
    cos = np.asarray(inputs['cos'], np.float32)[pos]
    sin = np.asarray(inputs['sin'], np.float32)[pos]
    sgn = np.concatenate([-np.ones(32, np.float32), np.ones(32, np.float32)])
    dd['cos_kT'] = np.ascontiguousarray(cos.T)
    dd['sin_kT'] = np.ascontiguousarray((sin * sgn[None, :]).T)
    dd['cos_qT'] = np.ascontiguousarray(cos[rows].T)
    dd['sin_qT'] = np.ascontiguousarray((sin[rows] * sgn[None, :]).T)
    q_pos = rows
    k_pos = np.arange(S)
    vis = (k_pos[:, None] <= q_pos[None, :]) \
        & (np.asarray(inputs['attention_mask'][b]) > 0)[:, None]
    dd['maskT'] = np.where(vis, 0.0, -1e30).astype(np.float32)
    return dd


def prep_in_maps(inputs):
    w = _prep_weights(inputs)
    in_maps = []
    for core in range(N_CORES):
        m = dict(w)
        m.update(_prep_core(inputs, core))
        in_maps.append(m)
    return in_maps


_NCS = {}


def _get_nc(loop_n=1):
    if loop_n not in _NCS:
        _NCS[loop_n] = build_nc(loop_n)
    return _NCS[loop_n]


_EXECS = {}   # loop_n -> (jitted_fn, in_names, out_names, out_avals, mesh)


def _get_exec(loop_n=1):
    """Build the 8-core sharded executable once (mirrors
    bass2jax.run_bass_via_pjrt's multi-core path, without donation so the
    callable can be re-invoked for timing)."""
    if loop_n not in _EXECS:
        import jax
        from jax.sharding import Mesh, PartitionSpec
        from jax.experimental.shard_map import shard_map
        import concourse.mybir as mybir_
        from concourse import bass2jax

        nc = _get_nc(loop_n)
        bass2jax.install_neuronx_cc_hook()
        pname = nc.partition_id_tensor.name if nc.partition_id_tensor else None
        in_names, out_names, out_avals = [], [], []
        for alloc in nc.m.functions[0].allocations:
            if not isinstance(alloc, mybir_.MemoryLocationSet):
                continue
            name = alloc.memorylocations[0].name
            if alloc.kind == "ExternalInput":
                if name != pname:
                    in_names.append(name)
            elif alloc.kind == "ExternalOutput":
                out_names.append(name)
                out_avals.append(jax.core.ShapedArray(
                    tuple(alloc.tensor_shape), mybir_.dt.np(alloc.dtype)))
        n_params = len(in_names)
        all_names = in_names + out_names
        if pname is not None:
            all_names = all_names + [pname]

        def _body(*args):
            operands = list(args)
            if pname is not None:
                operands.append(bass2jax.partition_id_tensor())
            outs = bass2jax._bass_exec_p.bind(
                *operands,
                out_avals=tuple(out_avals),
                in_names=tuple(all_names),
                out_names=tuple(out_names),
                lowering_input_output_aliases=(),
                sim_require_finite=True,
                sim_require_nnan=True,
                nc=nc,
            )
            return tuple(outs)

        devices = jax.devices()[:N_CORES]
        mesh = Mesh(np.asarray(devices), ("core",))
        nin = n_params + len(out_names)
        fn = jax.jit(shard_map(
            _body, mesh=mesh,
            in_specs=(PartitionSpec("core"),) * nin,
            out_specs=(PartitionSpec("core"),) * len(out_names),
            check_rep=False))
        _EXECS[loop_n] = (fn, in_names, out_names, out_avals, mesh)
    return _EXECS[loop_n]


def device_args(inputs, loop_n=1):
    """Concatenated (and device-put) arg list for the sharded executable."""
    import jax
    from jax.sharding import NamedSharding, PartitionSpec

    fn, in_names, out_names, out_avals, mesh = _get_exec(loop_n)
    in_maps = prep_in_maps(inputs)
    args = [np.concatenate([in_maps[c][n] for c in range(N_CORES)], axis=0)
            for n in in_names]
    for av in out_avals:
        args.append(np.zeros((N_CORES * av.shape[0],) + av.shape[1:], av.dtype))
    sh = NamedSharding(mesh, PartitionSpec("core"))
    return [jax.device_put(a, sh) for a in args]


def run(inputs):
    import jax

    fn, in_names, out_names, out_avals, mesh = _get_exec()
    args = device_args(inputs)
    outs = jax.block_until_ready(fn(*args))
    out_full = np.asarray(outs[0]).reshape(N_CORES, H, TQ)
    out = np.zeros((B, S, H), np.float32)
    for core in range(N_CORES):
        b, c = core // 4, core % 4
        out[b, _core_rows(c)] = out_full[core].T
    return out


def kernel(**inputs):
    return run(inputs)

